# revision 1
# baseline (speedup 1.0000x reference)
"""CausalPrefixAttention Trainium2 Bass kernel.

Sharding: core = 4*batch + head_group. Each core computes, for its batch b and
its 4 heads, the full pipeline LN(x), LN(context) -> q/k/v projections ->
causal-prefix attention -> out @ Wo_slice, producing a [2048, 1024] partial.
Host sums the 4 partials per batch (row-parallel Wo) and adds bo.
"""

import sys

import numpy as np

for _p in ("/opt/trn_rl_repo", "/root/.axon_site/_ro/trn_rl_repo"):
    if _p not in sys.path:
        sys.path.append(_p)

import ml_dtypes  # noqa: E402

import concourse.bass as bass  # noqa: E402
import concourse.mybir as mybir  # noqa: E402
import concourse.tile as tile  # noqa: E402
from concourse import bacc  # noqa: E402
from concourse.bass_utils import run_bass_kernel_spmd  # noqa: E402

BF16 = mybir.dt.bfloat16
F32 = mybir.dt.float32
F32R = mybir.dt.float32r

N = 2048          # query tokens per batch
CTX = 2048        # context tokens per batch
DIM = 1024
DH = 64           # head dim
HPC = 4           # heads per core
CPC = HPC * DH    # 256 inner cols per core
J = CTX + N       # 4096 total keys
EPS = 1e-5

AF = mybir.ActivationFunctionType
ALU = mybir.AluOpType


def build_nc() -> bass.Bass:
    nc = bacc.Bacc()

    xb = nc.declare_dram_parameter("xb", [N, DIM], BF16, isOutput=False)
    cb = nc.declare_dram_parameter("cb", [CTX, DIM], BF16, isOutput=False)
    wq = nc.declare_dram_parameter("wq", [DIM, CPC], BF16, isOutput=False)
    wk = nc.declare_dram_parameter("wk", [2, DIM, CPC], BF16, isOutput=False)
    wv = nc.declare_dram_parameter("wv", [2, DIM, CPC], BF16, isOutput=False)
    wo = nc.declare_dram_parameter("wo", [64, HPC, DIM], BF16, isOutput=False)
    cbq = nc.declare_dram_parameter("cbq", [128, 2], F32, isOutput=False)
    cbk = nc.declare_dram_parameter("cbk", [128, 2, 2], F32, isOutput=False)
    vbv = nc.declare_dram_parameter("vbv", [128, 2, HPC, 64], BF16,
                                    isOutput=False)
    cmv = nc.declare_dram_parameter("cmv", [128, 16], F32, isOutput=False)
    tri01 = nc.declare_dram_parameter("tri01", [128, 128], BF16, isOutput=False)
    ident = nc.declare_dram_parameter("ident", [128, 128], BF16, isOutput=False)
    sel16 = nc.declare_dram_parameter("sel16", [16, 2048], BF16, isOutput=False)
    out_d = nc.declare_dram_parameter("out", [N, DIM], F32, isOutput=True)

    with tile.TileContext(nc) as tc:
        with (
            tc.tile_pool(name="singles", bufs=1) as singles,
            tc.tile_pool(name="acts", bufs=1) as acts,
            tc.tile_pool(name="ln", bufs=6) as ln_pool,
            tc.tile_pool(name="lns", bufs=8) as lns,
            tc.tile_pool(name="es", bufs=4) as es_pool,
            tc.tile_pool(name="dstg", bufs=3) as dstg_pool,
            tc.tile_pool(name="outp", bufs=3) as out_pool,
            tc.tile_pool(name="ps", bufs=2, space="PSUM") as psum,
            tc.tile_pool(name="pj", bufs=1, space="PSUM") as pj_pool,
            tc.tile_pool(name="dnp", bufs=1, space="PSUM") as den_pool,
            tc.tile_pool(name="avps", bufs=2, space="PSUM") as av_pool,
        ):
            # --- constants / weights to SBUF ---
            wq_sb = singles.tile([128, 8, CPC], BF16)
            nc.gpsimd.dma_start(wq_sb, wq.rearrange("(t p) c -> p t c", p=128))
            wk_sb = singles.tile([128, 2, 8, CPC], BF16)
            nc.gpsimd.dma_start(wk_sb, wk.rearrange("s (t p) c -> p s t c", p=128))
            wv_sb = singles.tile([128, 2, 8, CPC], BF16)
            nc.gpsimd.dma_start(wv_sb, wv.rearrange("s (t p) c -> p s t c", p=128))
            wo_sb = singles.tile([64, HPC, DIM], BF16)
            nc.gpsimd.dma_start(wo_sb, wo[:])
            scol = singles.tile([128, 16, 16], BF16)
            nc.vector.memset(scol, 0.0)
            for r in range(16):
                nc.vector.memset(scol[64:65, r, r:r + 1], 1.0)
            cbq_sb = singles.tile([128, 2], F32)
            nc.sync.dma_start(cbq_sb, cbq[:])
            cbk_sb = singles.tile([128, 2, 2], F32)
            nc.sync.dma_start(cbk_sb, cbk[:])
            vb_sb = singles.tile([128, 2, HPC, 64], BF16)
            nc.sync.dma_start(vb_sb, vbv[:])
            cm_sb = singles.tile([128, 16], F32)
            nc.sync.dma_start(cm_sb, cmv[:])
            tri_sb = singles.tile([128, 128], BF16)
            nc.sync.dma_start(tri_sb, tri01[:])
            id_sb = singles.tile([128, 128], BF16)
            nc.sync.dma_start(id_sb, ident[:])
            sel_sb = singles.tile([16, 2048], BF16)
            nc.sync.dma_start(sel_sb, sel16[:])
            eps_sb = singles.tile([128, 1], F32)
            nc.vector.memset(eps_sb, EPS)

            # --- LayerNorm + transpose: [tokens, DIM] -> [128, 8, tokens] ---
            # stats on ScalarE via accum_out; apply + transpose-copy on DVE
            def layernorm_T(src_dram, ntok, name, ldeng):
                dstT = acts.tile([128, 8, ntok], BF16, tag=f"T{name}")
                for rt in range(ntok // 128):
                    xt = ln_pool.tile([128, DIM], BF16, tag="xt")
                    ldeng.dma_start(xt, src_dram[rt * 128:(rt + 1) * 128, :])
                    st = lns.tile([128, 2, 6], F32, tag="st")
                    nc.vector.bn_stats(st[:, 0, :], xt[:, 0:512])
                    nc.vector.bn_stats(st[:, 1, :], xt[:, 512:1024])
                    mv = lns.tile([128, 2], F32, tag="mv")
                    nc.vector.bn_aggr(mv, st)
                    std = lns.tile([128, 1], F32, tag="std")
                    nc.scalar.activation(std, mv[:, 1:2], AF.Sqrt, bias=eps_sb)
                    rstd = lns.tile([128, 1], F32, tag="rstd")
                    nc.vector.reciprocal(rstd, std)
                    xn = ln_pool.tile([128, DIM], BF16, tag="xn")
                    nc.vector.tensor_scalar(
                        xn, xt, mv[:, 0:1], rstd, op0=ALU.subtract, op1=ALU.mult
                    )
                    for fg in range(2):
                        pst = av_pool.tile([128, 512], BF16, tag="av")
                        for k in range(4):
                            ft = fg * 4 + k
                            nc.tensor.transpose(
                                pst[:, k * 128:(k + 1) * 128],
                                xn[:, ft * 128:(ft + 1) * 128],
                                id_sb,
                            )
                        nc.scalar.copy(
                            dstT[:, fg * 4:(fg + 1) * 4, rt * 128:(rt + 1) * 128],
                            pst.rearrange("p (f c) -> p f c", f=4),
                        )
                return dstT

            xnT = layernorm_T(xb, N, "x", nc.sync)
            cnT = layernorm_T(cb, CTX, "c", nc.gpsimd)

            # --- projections, split per c-tile so attention on heads 0/1 can
            # overlap with the projections for heads 2/3 ---
            def make_qT(ct):
                chunks = []
                for it in range(N // 1024):
                    qT = acts.tile([128, 1024], BF16, tag=f"qT{ct}_{it}",
                                   name=f"qT{ct}_{it}")
                    for half in range(2):
                        ps = pj_pool.tile([128, 512], F32, tag="pj")
                        off = it * 1024 + half * 512
                        for kt in range(8):
                            nc.tensor.matmul(
                                ps,
                                wq_sb[:, kt, ct * 128:(ct + 1) * 128],
                                xnT[:, kt, off:off + 512],
                                start=(kt == 0), stop=(kt == 7),
                            )
                        nc.vector.tensor_scalar_add(
                            qT[:, half * 512:(half + 1) * 512], ps,
                            cbq_sb[:, ct:ct + 1])
                    chunks.append(qT)
                return chunks

            def make_kT(ct):
                chunks = []
                for jt in range(J // 1024):
                    kT = acts.tile([128, 1024], BF16, tag=f"kT{ct}_{jt}",
                                   name=f"kT{ct}_{jt}")
                    for half in range(2):
                        ps = pj_pool.tile([128, 512], F32, tag="pj")
                        j5 = jt * 2 + half
                        s = 0 if j5 < 4 else 1
                        srcT = cnT if j5 < 4 else xnT
                        off = (j5 % 4) * 512
                        for kt in range(8):
                            nc.tensor.matmul(
                                ps,
                                wk_sb[:, s, kt, ct * 128:(ct + 1) * 128],
                                srcT[:, kt, off:off + 512],
                                start=(kt == 0), stop=(kt == 7),
                            )
                        nc.vector.tensor_scalar_add(
                            kT[:, half * 512:(half + 1) * 512], ps,
                            cbk_sb[:, s, ct:ct + 1])
                    chunks.append(kT)
                return chunks

            qTs = {0: make_qT(0)}
            kTs = {0: make_kT(0)}

            # --- v natural [keys, 4 heads, 64+aug] ---
            v_tiles = []
            for jb in range(32):
                vt = acts.tile([128, HPC, 66], BF16, tag=f"v{jb}",
                               name=f"v{jb}")
                v_tiles.append(vt)
                s = 0 if jb < 16 else 1
                srcT = cnT if jb < 16 else xnT
                off = (jb % 16) * 128
                ps = pj_pool.tile([128, 512], F32, tag="pj")
                for kt in range(8):
                    nc.tensor.matmul(
                        ps[:, 0:CPC],
                        srcT[:, kt, off:off + 128],
                        wv_sb[:, s, kt, :],
                        start=(kt == 0), stop=(kt == 7),
                    )
                nc.vector.tensor_add(
                    vt[:, :, 0:64],
                    ps[:, 0:CPC].rearrange("p (h d) -> p h d", h=HPC),
                    vb_sb[:, s, :, :],
                )
                if jb < 16:
                    # context_mask: zero masked rows, aug col = mask
                    nc.vector.tensor_scalar_mul(
                        vt[:, :, 0:64], vt[:, :, 0:64],
                        cm_sb[:, jb:jb + 1],
                    )
                    nc.vector.tensor_copy(
                        vt[:, :, 64:65],
                        cm_sb[:, jb:jb + 1, None].to_broadcast((128, HPC, 1)),
                    )
                else:
                    nc.vector.memset(vt[:, :, 64:65], 1.0)

            # --- attention ---
            rden = singles.tile([16, 512], BF16)
            # out^T as 16 separate tiles (head, 512-query block) so the out
            # projection can start per-block as soon as normalization lands
            oThs = {}
            for h in range(HPC):
                for q in range(4):
                    oThs[(h, q)] = acts.tile([128, 512], BF16, tag=f"oT{h}_{q}", name=f"oT{h}_{q}")
            den_acc = den_pool.tile([8, 512], F32)
            n_den = [0, 0]

            def attend(h):
                ct, pb = h // 2, (h % 2) * 64
                kT, qT = kTs[ct], qTs[ct]
                for it in range(2):
                    i0 = it * 1024
                    njs = 16 + it * 8 + 8
                    jl0 = [j for j in range(njs)
                           if j < 16 or (j - 16) * 128 - i0 < 512]
                    jl1 = list(range(njs))
                    av0 = av_pool.tile([128, 512], F32, tag="av")
                    av1 = av_pool.tile([128, 512], F32, tag="av")

                    def emit_av(jb, es):
                        if jb in jl0:
                            nc.tensor.matmul(
                                av0[0:65, :],
                                v_tiles[jb][:, h, 0:65],
                                es[:, 0:512],
                                start=(jb == jl0[0]), stop=(jb == jl0[-1]),
                            )
                        nc.tensor.matmul(
                            av1[0:65, :],
                            v_tiles[jb][:, h, 0:65],
                            es[:, 512:1024],
                            start=(jb == jl1[0]), stop=(jb == jl1[-1]),
                        )

                    pending = None  # (jb, es) awaiting AV, lags one j-block
                    for jb in range(njs):
                        jj0 = (jb - 16) * 128
                        d = jj0 - i0
                        crossing = jb >= 16 and d >= 0
                        c0 = d if (crossing and d > 0) else 0
                        kc = kT[jb // 8][pb:pb + 64,
                                         (jb % 8) * 128:(jb % 8 + 1) * 128]
                        qc = qT[it]
                        ps = psum.tile([128, 1024], F32, tag="ps")
                        if c0 < 512:
                            nc.tensor.matmul(
                                ps[:, c0:512],
                                kc,
                                qc[pb:pb + 64, c0:512],
                                start=True, stop=True,
                            )
                        nc.tensor.matmul(
                            ps[:, max(512, c0):1024],
                            kc,
                            qc[pb:pb + 64, max(512, c0):1024],
                            start=True, stop=True,
                        )
                        es = es_pool.tile([128, 1024], BF16, tag="es")
                        if c0 > 0:
                            nc.gpsimd.memset(es[:, 0:c0], 0.0)
                        nc.scalar.activation(
                            es[:, c0:1024], ps[:, c0:1024], AF.Exp)
                        if crossing:
                            nc.gpsimd.tensor_mul(
                                es[:, d:d + 128], es[:, d:d + 128], tri_sb
                            )
                        if pending is not None:
                            emit_av(*pending)
                        pending = (jb, es)
                    emit_av(*pending)
                    for half, av in ((0, av0), (1, av1)):
                        i5 = i0 + half * 512
                        nc.vector.tensor_copy(
                            oThs[(h, i5 // 512)][0:64, :], av[0:64, :])
                        dstg = dstg_pool.tile([128, 512], BF16, tag="dstg")
                        nc.vector.tensor_copy(dstg[64:65, :], av[64:65, :])
                        r = h * 4 + it * 2 + half
                        grp = r // 8
                        n_den[grp] += 1
                        nc.tensor.matmul(
                            den_acc,
                            scol[64:65, r, grp * 8:(grp + 1) * 8],
                            dstg[64:65, :],
                            start=(n_den[grp] == 1), stop=(n_den[grp] == 8),
                        )

            def normalize(heads, rd):
                for it in range(4):
                    for h in heads:
                        r = (h % 2) * 4 + it
                        bc = av_pool.tile([128, 512], F32, tag="av")
                        nc.tensor.matmul(
                            bc,
                            sel_sb[0:8, r * 128:(r + 1) * 128],
                            rd,
                            start=True, stop=True,
                        )
                        nc.vector.tensor_mul(
                            oThs[(h, it)][0:64, :],
                            oThs[(h, it)][0:64, :], bc[0:64, :],
                        )

            attend(0)
            qTs[1] = make_qT(1)
            kTs[1] = make_kT(1)
            attend(1)
            with nc.allow_low_precision(reason="bf16 denom broadcast"):
                nc.vector.reciprocal(rden[0:8, :], den_acc)
            normalize([0, 1], rden[0:8, :])
            attend(2)
            attend(3)
            rden2 = singles.tile([8, 512], BF16)
            with nc.allow_low_precision(reason="bf16 denom broadcast"):
                nc.vector.reciprocal(rden2[0:8, :], den_acc)
            normalize([2, 3], rden2[0:8, :])

            # --- out projection: out[i, :] = oTh^T @ wo (4 heads, K=64) ---
            for ib in range(N // 128):
                ot = out_pool.tile([128, DIM], F32, tag="ot")
                ps = psum.tile([128, 1024], F32, tag="ps")
                for oc in range(2):
                    for h in range(HPC):
                        nc.tensor.matmul(
                            ps[:, oc * 512:(oc + 1) * 512],
                            oThs[(h, ib // 4)][0:64,
                                               (ib % 4) * 128:(ib % 4 + 1) * 128],
                            wo_sb[:, h, oc * 512:(oc + 1) * 512],
                            start=(h == 0), stop=(h == 3),
                        )
                nc.scalar.copy(ot, ps)
                nc.sync.dma_start(out_d[ib * 128:(ib + 1) * 128, :], ot)

    nc.finalize()
    return nc


def _sel16():
    s = np.zeros((16, 2048), np.float32)
    for r in range(16):
        s[r, r * 128:(r + 1) * 128] = 1.0
    return s


def make_in_maps(x, context, context_mask, g1, b1, g2, b2, Wq, Wkv, Wo):
    bf = ml_dtypes.bfloat16
    Wk = Wkv[:, :DIM]
    Wv = Wkv[:, DIM:]
    scale = DH ** -0.5
    tri = np.triu(np.ones((128, 128), np.float32)).astype(bf)
    g1 = np.asarray(g1, np.float32)
    g2 = np.asarray(g2, np.float32)
    b1 = np.asarray(b1, np.float32)
    b2 = np.asarray(b2, np.float32)

    in_maps = []
    for core in range(8):
        b, g = core // 4, core % 4
        hs = slice(g * CPC, (g + 1) * CPC)
        wq_g = g1[:, None] * Wq[:, hs] * scale
        # source 0 = context (g2/b2), source 1 = self (g1/b1)
        wk2 = np.stack([g2[:, None] * Wk[:, hs], g1[:, None] * Wk[:, hs]])
        wv2 = np.stack([g2[:, None] * Wv[:, hs], g1[:, None] * Wv[:, hs]])
        cbq_a = (b1 @ Wq[:, hs]) * scale          # [256]
        cbk_a = np.stack([b2 @ Wk[:, hs], b1 @ Wk[:, hs]])   # [2, 256]
        vb_a = np.stack([b2 @ Wv[:, hs], b1 @ Wv[:, hs]])    # [2, 256]
        in_maps.append(dict(
            xb=np.ascontiguousarray(x[b]).astype(bf),
            cb=np.ascontiguousarray(context[b]).astype(bf),
            wq=np.ascontiguousarray(wq_g).astype(bf),
            wk=np.ascontiguousarray(wk2).astype(bf),
            wv=np.ascontiguousarray(wv2).astype(bf),
            wo=np.ascontiguousarray(
                Wo[hs, :].reshape(HPC, 64, DIM).transpose(1, 0, 2)
            ).astype(bf),
            cbq=np.ascontiguousarray(cbq_a.reshape(2, 128).T),
            cbk=np.ascontiguousarray(
                cbk_a.reshape(2, 2, 128).transpose(2, 0, 1)),
            vbv=np.ascontiguousarray(np.broadcast_to(
                vb_a.reshape(1, 2, HPC, 64), (128, 2, HPC, 64))).astype(bf),
            cmv=np.ascontiguousarray(
                np.asarray(context_mask[b], np.float32).reshape(16, 128).T
            ),
            tri01=tri,
            ident=np.eye(128, dtype=np.float32).astype(bf),
            sel16=_sel16().astype(bf),
        ))
    return in_maps


_NC_CACHE = None


def kernel(**inputs) -> np.ndarray:
    global _NC_CACHE
    x = np.asarray(inputs["x"], np.float32)
    context = np.asarray(inputs["context"], np.float32)
    cm = np.asarray(inputs["context_mask"])
    g1 = np.asarray(inputs["g1"], np.float32)
    b1 = np.asarray(inputs["b1"], np.float32)
    g2 = np.asarray(inputs["g2"], np.float32)
    b2 = np.asarray(inputs["b2"], np.float32)
    Wq = np.asarray(inputs["Wq"], np.float32)
    Wkv = np.asarray(inputs["Wkv"], np.float32)
    Wo = np.asarray(inputs["Wo"], np.float32)
    bo = np.asarray(inputs["bo"], np.float32)

    if _NC_CACHE is None:
        _NC_CACHE = build_nc()
    nc = _NC_CACHE

    # The SPMD run dispatches through jax/PJRT on the axon backend; if the
    # caller pinned jax to cpu (common for reference computation), restore
    # the full platform list so the 8 NeuronCores are visible.
    import jax
    if len(jax.devices()) < 8:
        import os
        os.environ.pop("JAX_PLATFORMS", None)
        try:
            jax.config.update("jax_platforms", None)
        except Exception:
            pass
        try:
            from jax.extend import backend as _jxb
            _jxb.clear_backends()
        except Exception:
            from jax._src import xla_bridge as _xb
            _xb.backends.cache_clear()

    in_maps = make_in_maps(x, context, cm, g1, b1, g2, b2, Wq, Wkv, Wo)
    res = run_bass_kernel_spmd(nc, in_maps, core_ids=list(range(8))).results

    out = np.zeros((2, N, DIM), np.float32)
    for core in range(8):
        out[core // 4] += np.asarray(res[core]["out"], np.float32)
    out += bo
    return out



# revision 8
# speedup vs baseline: 1.1629x; 1.1629x over previous
"""CausalPrefixAttention Trainium2 Bass kernel (v2).

Sharding: core = 4*batch + head_group (2 heads-pairs = 4 heads per core).
Per core: LN(x), LN(context) -> q/k/v projections -> causal-prefix attention
-> out @ Wo_slice, producing a [2048, 1024] partial. Host sums the 4 partials
per batch (row-parallel Wo) and adds bo.

v2 redesign vs baseline:
- AV matmul emits attnout^T [queries, dh] per 128-query block (N=65 per
  key-block instead of 512): halves AV tensor-engine time and makes the
  softmax denominator a per-partition scalar (normalize on DVE, no PE
  broadcast matmuls).
- 4 accumulation regions packed per PSUM bank using the pending-zero
  region: only region 0 issues start=True.
- Head-pair packed attnout transpose + K=128 out-projection.
- LN: stats on DVE, context apply/copies on Pool, x tiles 8..15 deferred.
- Deadline-ordered filler injection keeps PE busy during the ACT(exp)-bound
  attention phases.
"""

import sys

import numpy as np

for _p in ("/opt/trn_rl_repo", "/root/.axon_site/_ro/trn_rl_repo"):
    if _p not in sys.path:
        sys.path.append(_p)

import ml_dtypes  # noqa: E402

import concourse.bass as bass  # noqa: E402
import concourse.mybir as mybir  # noqa: E402
import concourse.tile as tile  # noqa: E402
from concourse import bacc  # noqa: E402
from concourse.bass_utils import run_bass_kernel_spmd  # noqa: E402

BF16 = mybir.dt.bfloat16
F32 = mybir.dt.float32

N = 2048          # query tokens per batch
CTX = 2048        # context tokens per batch
DIM = 1024
DH = 64           # head dim
HPC = 4           # heads per core
CPC = HPC * DH    # 256 inner cols per core
J = CTX + N       # 4096 total keys
EPS = 1e-5

AF = mybir.ActivationFunctionType
ALU = mybir.AluOpType


def build_nc() -> bass.Bass:
    nc = bacc.Bacc()

    xb = nc.declare_dram_parameter("xb", [N, DIM], BF16, isOutput=False)
    cb = nc.declare_dram_parameter("cb", [CTX, DIM], BF16, isOutput=False)
    wq = nc.declare_dram_parameter("wq", [DIM, CPC], BF16, isOutput=False)
    wk = nc.declare_dram_parameter("wk", [2, DIM, CPC], BF16, isOutput=False)
    wv = nc.declare_dram_parameter("wv", [2, DIM, CPC], BF16, isOutput=False)
    wo = nc.declare_dram_parameter("wo", [2, 128, DIM], BF16, isOutput=False)
    cbq = nc.declare_dram_parameter("cbq", [128, 2], F32, isOutput=False)
    cbk = nc.declare_dram_parameter("cbk", [128, 2, 2], F32, isOutput=False)
    vbv = nc.declare_dram_parameter("vbv", [128, 2, HPC, 64], BF16,
                                    isOutput=False)
    cmv = nc.declare_dram_parameter("cmv", [128, 16], F32, isOutput=False)
    tri01 = nc.declare_dram_parameter("tri01", [128, 128], BF16, isOutput=False)
    ident = nc.declare_dram_parameter("ident", [128, 128], BF16, isOutput=False)
    out_d = nc.declare_dram_parameter("out", [N, DIM], F32, isOutput=True)

    with tile.TileContext(nc) as tc:
        with (
            tc.tile_pool(name="singles", bufs=1) as singles,
            tc.tile_pool(name="acts", bufs=1) as acts,
            tc.tile_pool(name="ln", bufs=4) as ln_pool,
            tc.tile_pool(name="lns", bufs=8) as lns,
            tc.tile_pool(name="es", bufs=6) as es_pool,
            tc.tile_pool(name="ott", bufs=12) as ott_pool,
            tc.tile_pool(name="rcp", bufs=8) as rcp_pool,
            tc.tile_pool(name="outp", bufs=3) as out_pool,
            tc.tile_pool(name="ps", bufs=2, space="PSUM") as ps_pool,
            tc.tile_pool(name="ch", bufs=3, space="PSUM") as ch_pool,
            tc.tile_pool(name="pj", bufs=1, space="PSUM") as pj_pool,
        ):
            # --- weights / constants to SBUF (gpsimd DMA queue) ---
            wq_sb = singles.tile([128, 8, CPC], BF16)
            nc.gpsimd.dma_start(wq_sb, wq.rearrange("(t p) c -> p t c", p=128))
            wk_sb = singles.tile([128, 2, 8, CPC], BF16)
            nc.gpsimd.dma_start(wk_sb, wk.rearrange("s (t p) c -> p s t c", p=128))
            wv_sb = singles.tile([128, 2, 8, CPC], BF16)
            nc.gpsimd.dma_start(wv_sb, wv.rearrange("s (t p) c -> p s t c", p=128))
            wo_sb = singles.tile([128, 2, DIM], BF16)
            nc.gpsimd.dma_start(wo_sb, wo.rearrange("g p c -> p g c"))
            cbq_sb = singles.tile([128, 2], F32)
            nc.gpsimd.dma_start(cbq_sb, cbq[:])
            cbk_sb = singles.tile([128, 2, 2], F32)
            nc.gpsimd.dma_start(cbk_sb, cbk[:])
            vb_sb = singles.tile([128, 2, HPC, 64], BF16)
            nc.gpsimd.dma_start(vb_sb, vbv[:])
            cm_sb = singles.tile([128, 16], F32)
            nc.gpsimd.dma_start(cm_sb, cmv[:])
            tri_sb = singles.tile([128, 128], BF16)
            nc.gpsimd.dma_start(tri_sb, tri01[:])
            id_sb = singles.tile([128, 128], BF16)
            nc.gpsimd.dma_start(id_sb, ident[:])
            eps_sb = singles.tile([128, 1], F32)
            nc.vector.memset(eps_sb, EPS)

            # --- persistent activation tiles ---
            xnT = acts.tile([128, 8, N], BF16, tag="Tx")
            cnT = acts.tile([128, 8, CTX], BF16, tag="Tc")
            qT = {}
            for ct in range(2):
                for it in range(2):
                    qT[(ct, it)] = acts.tile([128, 1024], BF16,
                                             tag=f"qT{ct}{it}",
                                             name=f"qT{ct}{it}")
            kT = {}
            for ct in range(2):
                for jt in range(4):
                    kT[(ct, jt)] = acts.tile([128, 1024], BF16,
                                             tag=f"kT{ct}{jt}",
                                             name=f"kT{ct}{jt}")
            v_tiles = []
            for jb in range(32):
                v_tiles.append(acts.tile([128, HPC, 66], BF16, tag=f"v{jb}",
                                         name=f"v{jb}"))
            oT = {g: acts.tile([128, 2048], BF16, tag=f"oT{g}",
                               name=f"oT{g}") for g in range(2)}

            # --- LayerNorm + transpose one 128-row tile ---
            # stats always DVE (bn_stats is DVE-only); apply/copy routable.
            def emit_ln_tile(src, dstT, rt, apply_eng, copy_eng, psum_pool,
                             nm):
                xt = ln_pool.tile([128, DIM], BF16, tag="xt",
                                  name=f"xt{nm}{rt}")
                nc.sync.dma_start(xt, src[rt * 128:(rt + 1) * 128, :])
                st = lns.tile([128, 2, 6], F32, tag="st", name=f"st{nm}{rt}")
                nc.vector.bn_stats(st[:, 0, :], xt[:, 0:512])
                nc.vector.bn_stats(st[:, 1, :], xt[:, 512:1024])
                mv = lns.tile([128, 2], F32, tag="mv", name=f"mv{nm}{rt}")
                nc.vector.bn_aggr(mv, st)
                std = lns.tile([128, 1], F32, tag="std", name=f"sd{nm}{rt}")
                nc.scalar.activation(std, mv[:, 1:2], AF.Sqrt, bias=eps_sb)
                rstd = lns.tile([128, 1], F32, tag="rstd", name=f"rs{nm}{rt}")
                nc.vector.reciprocal(rstd, std)
                xn = ln_pool.tile([128, DIM], BF16, tag="xn",
                                  name=f"xn{nm}{rt}")
                apply_eng.tensor_scalar(
                    xn, xt, mv[:, 0:1], rstd, op0=ALU.subtract, op1=ALU.mult
                )
                for fg in range(2):
                    pst = psum_pool.tile(
                        [128, 512], BF16, tag="ps" if psum_pool is ps_pool
                        else "pj", name=f"pst{nm}{rt}{fg}")
                    for k in range(4):
                        nc.tensor.transpose(
                            pst[:, k * 128:(k + 1) * 128],
                            xn[:, (fg * 4 + k) * 128:(fg * 4 + k + 1) * 128],
                            id_sb,
                        )
                    dst = dstT[:, fg * 4:(fg + 1) * 4, rt * 128:(rt + 1) * 128]
                    src = pst.rearrange("p (f c) -> p f c", f=4)
                    if hasattr(copy_eng, "tensor_copy"):
                        copy_eng.tensor_copy(dst, src)
                    else:
                        copy_eng.copy(dst, src)

            # --- projection emitters (one PSUM half each) ---
            def emit_qproj(ct, it, half):
                ps = pj_pool.tile([128, 512], F32, tag="pj",
                                  name=f"qp{ct}{it}{half}")
                off = it * 1024 + half * 512
                for kt in range(8):
                    nc.tensor.matmul(
                        ps,
                        wq_sb[:, kt, ct * 128:(ct + 1) * 128],
                        xnT[:, kt, off:off + 512],
                        start=(kt == 0), stop=(kt == 7),
                    )
                nc.vector.tensor_scalar_add(
                    qT[(ct, it)][:, half * 512:(half + 1) * 512], ps,
                    cbq_sb[:, ct:ct + 1])

            def emit_kproj(ct, jt, half):
                ps = pj_pool.tile([128, 512], F32, tag="pj",
                                  name=f"kp{ct}{jt}{half}")
                j5 = jt * 2 + half
                s = 0 if j5 < 4 else 1
                srcT = cnT if j5 < 4 else xnT
                off = (j5 % 4) * 512
                for kt in range(8):
                    nc.tensor.matmul(
                        ps,
                        wk_sb[:, s, kt, ct * 128:(ct + 1) * 128],
                        srcT[:, kt, off:off + 512],
                        start=(kt == 0), stop=(kt == 7),
                    )
                nc.vector.tensor_scalar_add(
                    kT[(ct, jt)][:, half * 512:(half + 1) * 512], ps,
                    cbk_sb[:, s, ct:ct + 1])

            def emit_vproj(jb):
                vt = v_tiles[jb]
                s = 0 if jb < 16 else 1
                srcT = cnT if jb < 16 else xnT
                off = (jb % 16) * 128
                ps = pj_pool.tile([128, 512], F32, tag="pj", name=f"vp{jb}")
                for kt in range(8):
                    nc.tensor.matmul(
                        ps[:, 0:CPC],
                        srcT[:, kt, off:off + 128],
                        wv_sb[:, s, kt, :],
                        start=(kt == 0), stop=(kt == 7),
                    )
                nc.vector.tensor_add(
                    vt[:, :, 0:64],
                    ps[:, 0:CPC].rearrange("p (h d) -> p h d", h=HPC),
                    vb_sb[:, s, :, :],
                )
                if jb < 16:
                    nc.vector.tensor_scalar_mul(
                        vt[:, :, 0:64], vt[:, :, 0:64], cm_sb[:, jb:jb + 1])
                    nc.vector.tensor_copy(
                        vt[:, :, 64:65],
                        cm_sb[:, jb:jb + 1, None].to_broadcast((128, HPC, 1)),
                    )
                else:
                    nc.vector.memset(vt[:, :, 64:65], 1.0)

            # --- out projection for one 128-row block ---
            def emit_outproj(ib):
                ot = out_pool.tile([128, DIM], F32, tag="ot", name=f"ot{ib}")
                for oc in range(2):
                    pso = ch_pool.tile([128, 512], F32, tag="ch",
                                       name=f"po{ib}{oc}")
                    for g in range(2):
                        nc.tensor.matmul(
                            pso,
                            oT[g][:, ib * 128:(ib + 1) * 128],
                            wo_sb[:, g, oc * 512:(oc + 1) * 512],
                            start=(g == 0), stop=(g == 1),
                        )
                    # Pool cannot read PSUM: only ACT / DVE for these copies
                    if (2 * ib + oc) % 2 == 0:
                        nc.scalar.copy(ot[:, oc * 512:(oc + 1) * 512], pso)
                    else:
                        nc.vector.tensor_copy(
                            ot[:, oc * 512:(oc + 1) * 512], pso)
                nc.sync.dma_start(out_d[ib * 128:(ib + 1) * 128, :], ot)

            # --- deadline-ordered filler queue ---
            # entries: (phase_idx, kb_deadline, thunk); phases numbered 0..7
            # in execution order P00,P10,P20,P30,P01,P11,P21,P31.
            fillers = []

            def pump(phase_idx, kb):
                while fillers and fillers[0][0] * 100 + fillers[0][1] <= (
                        phase_idx * 100 + kb + 4):
                    fillers.pop(0)[2]()
                # steady drip: one thunk every 3rd kb if next deadline is
                # within the next two phases
                if kb % 3 == 2 and fillers and fillers[0][0] <= phase_idx + 2:
                    fillers.pop(0)[2]()

            # --- attention phase: one head, one 1024-query block ---
            ott_tiles = {}

            def attend(h, it, phase_idx, inline_stream=None):
                ct, pb = h // 2, (h % 2) * 64
                g = h // 2
                qc = qT[(ct, it)]
                njs = 24 if it == 0 else 32
                chA = ch_pool.tile([128, 512], F32, tag="ch",
                                   name=f"chA{h}{it}")
                chB = ch_pool.tile([128, 512], F32, tag="ch",
                                   name=f"chB{h}{it}")

                def region(rl):
                    cht = chA if rl < 4 else chB
                    r = rl % 4
                    return cht[:, r * 65:(r + 1) * 65]

                def emit_av(kb, es, c0):
                    first = (kb == 0)
                    for rl in range(8):
                        if rl * 128 < c0:
                            continue
                        stop_kb = (16 + rl) if it == 0 else (24 + rl)
                        reg = region(rl)
                        nc.tensor.matmul(
                            reg,
                            es[:, rl * 128:(rl + 1) * 128],
                            v_tiles[kb][:, h, 0:65],
                            start=(first and rl % 4 == 0),
                            stop=(kb == stop_kb),
                            skip_group_check=True,
                        )
                        if kb == stop_kb:
                            rcp = rcp_pool.tile([128, 1], F32, tag="rcp",
                                                name=f"rc{h}{it}{rl}")
                            nc.vector.reciprocal(rcp, reg[:, 64:65])
                            qg = it * 8 + rl
                            key = (g, qg)
                            if key not in ott_tiles:
                                ott_tiles[key] = ott_pool.tile(
                                    [128, 128], BF16, tag="ott",
                                    name=f"ott{g}_{qg}")
                            nc.vector.tensor_scalar_mul(
                                ott_tiles[key][:, pb:pb + 64],
                                reg[:, 0:64], rcp)

                pending = None
                for kb in range(njs):
                    d = (kb - 16) * 128 - it * 1024
                    crossing = kb >= 16 and d >= 0
                    c0 = d if (crossing and d > 0) else 0
                    kc = kT[(ct, kb // 8)][pb:pb + 64,
                                           (kb % 8) * 128:(kb % 8 + 1) * 128]
                    ps = ps_pool.tile([128, 1024], F32, tag="ps",
                                      name=f"sim{h}{it}{kb}")
                    if c0 < 512:
                        nc.tensor.matmul(ps[:, c0:512], kc,
                                         qc[pb:pb + 64, c0:512],
                                         start=True, stop=True)
                    nc.tensor.matmul(ps[:, max(512, c0):1024], kc,
                                     qc[pb:pb + 64, max(512, c0):1024],
                                     start=True, stop=True)
                    es = es_pool.tile([128, 1024], BF16, tag="es",
                                      name=f"es{h}{it}{kb}")
                    nc.scalar.activation(es[:, c0:1024], ps[:, c0:1024],
                                         AF.Exp)
                    if crossing:
                        nc.gpsimd.tensor_mul(es[:, d:d + 128],
                                             es[:, d:d + 128], tri_sb)
                    if pending is not None:
                        emit_av(*pending)
                    pending = (kb, es, c0)
                    if inline_stream is not None:
                        inline_stream(kb)
                    pump(phase_idx, kb)
                emit_av(*pending)

            # transpose staged attnout pairs into oT[g] for one it-block
            def emit_ott_transposes(g, it):
                for rl in range(8):
                    qg = it * 8 + rl
                    ott = ott_tiles[(g, qg)]
                    pst = pj_pool.tile([128, 128], BF16, tag="pj",
                                       name=f"otr{g}{qg}")
                    nc.tensor.transpose(pst, ott, id_sb)
                    nc.vector.tensor_copy(
                        oT[g][:, qg * 128:(qg + 1) * 128], pst)

            # ================= emission schedule =================
            # LEAD: ctx tiles 0..7 (apply/copy on Pool), x tiles 0..7 (DVE
            # apply, ACT copy), first projections. ctx 8..15 streams inside
            # P00 (2-kb lookahead ahead of the v projections that need it).
            for i in range(4):
                emit_ln_tile(cb, cnT, i, nc.gpsimd, nc.scalar, ps_pool, "c")
            for i in range(8):
                emit_ln_tile(xb, xnT, i, nc.vector, nc.scalar, ps_pool, "x")
            for i in range(4, 8):
                emit_ln_tile(cb, cnT, i, nc.gpsimd, nc.scalar, ps_pool, "c")
            emit_qproj(0, 0, 0)
            emit_qproj(0, 0, 1)
            emit_kproj(0, 0, 0)
            emit_kproj(0, 0, 1)
            for jb in range(4):
                emit_vproj(jb)

            # P00 inline stream: ctx LN 8..15, v kb+2, k jt1/jt2 halves
            def p00_stream(kb):
                if kb < 8:
                    emit_ln_tile(cb, cnT, 8 + kb, nc.gpsimd, nc.vector,
                                 pj_pool, "c")
                if 4 <= kb + 2 <= 23:
                    emit_vproj(kb + 2)
                if kb == 5:
                    emit_kproj(0, 1, 0)
                elif kb == 9:
                    emit_kproj(0, 1, 1)
                elif kb == 11:
                    emit_kproj(0, 2, 0)
                elif kb == 13:
                    emit_kproj(0, 2, 1)

            attend(0, 0, 0, inline_stream=p00_stream)

            # fillers with deadlines (phase_idx, kb)
            fillers.extend([
                (1, 4, lambda: emit_qproj(1, 0, 0)),
                (1, 10, lambda: emit_qproj(1, 0, 1)),
                (1, 16, lambda: emit_kproj(1, 0, 0)),
                (1, 22, lambda: emit_kproj(1, 0, 1)),
                (2, 6, lambda: emit_kproj(1, 1, 0)),
                (2, 10, lambda: emit_kproj(1, 1, 1)),
                (2, 14, lambda: emit_kproj(1, 2, 0)),
                (2, 18, lambda: emit_kproj(1, 2, 1)),
            ])
            for i in range(8):
                rt = 8 + i
                fillers.append(
                    (3, 2 + 2 * i,
                     lambda rt=rt: emit_ln_tile(xb, xnT, rt, nc.vector,
                                                nc.vector, pj_pool, "x")))
            fillers.extend([
                (3, 19, lambda: emit_qproj(0, 1, 0)),
                (3, 22, lambda: emit_qproj(0, 1, 1)),
                (4, 6, lambda: emit_kproj(0, 3, 0)),
                (4, 12, lambda: emit_kproj(0, 3, 1)),
                (4, 16, lambda: emit_vproj(24)),
                (4, 18, lambda: emit_vproj(25)),
                (4, 20, lambda: emit_vproj(26)),
                (4, 22, lambda: emit_vproj(27)),
                (4, 24, lambda: emit_vproj(28)),
                (4, 26, lambda: emit_vproj(29)),
                (4, 28, lambda: emit_vproj(30)),
                (4, 29, lambda: emit_vproj(31)),
                (5, 8, lambda: emit_qproj(1, 1, 0)),
                (5, 14, lambda: emit_qproj(1, 1, 1)),
                (5, 20, lambda: emit_kproj(1, 3, 0)),
                (5, 26, lambda: emit_kproj(1, 3, 1)),
            ])

            attend(1, 0, 1)
            emit_ott_transposes(0, 0)
            attend(2, 0, 2)
            attend(3, 0, 3)
            emit_ott_transposes(1, 0)

            # out-proj for it0 rows becomes available now; spread over the
            # it1 phases via fillers
            for ib in range(8):
                ph = 5 + ib // 4
                fillers.append((ph, 4 + 6 * (ib % 4),
                                lambda ib=ib: emit_outproj(ib)))

            attend(0, 1, 4)
            attend(1, 1, 5)
            emit_ott_transposes(0, 1)
            attend(2, 1, 6)
            attend(3, 1, 7)
            emit_ott_transposes(1, 1)

            # drain remaining fillers, then tail out-proj
            for _, _, th in fillers:
                th()
            fillers.clear()
            for ib in range(8, 16):
                emit_outproj(ib)

    nc.finalize()
    return nc


def make_in_maps(x, context, context_mask, g1, b1, g2, b2, Wq, Wkv, Wo):
    bf = ml_dtypes.bfloat16
    Wk = Wkv[:, :DIM]
    Wv = Wkv[:, DIM:]
    scale = DH ** -0.5
    tri = np.triu(np.ones((128, 128), np.float32)).astype(bf)
    g1 = np.asarray(g1, np.float32)
    g2 = np.asarray(g2, np.float32)
    b1 = np.asarray(b1, np.float32)
    b2 = np.asarray(b2, np.float32)

    in_maps = []
    for core in range(8):
        b, g = core // 4, core % 4
        hs = slice(g * CPC, (g + 1) * CPC)
        wq_g = g1[:, None] * Wq[:, hs] * scale
        # source 0 = context (g2/b2), source 1 = self (g1/b1)
        wk2 = np.stack([g2[:, None] * Wk[:, hs], g1[:, None] * Wk[:, hs]])
        wv2 = np.stack([g2[:, None] * Wv[:, hs], g1[:, None] * Wv[:, hs]])
        cbq_a = (b1 @ Wq[:, hs]) * scale          # [256]
        cbk_a = np.stack([b2 @ Wk[:, hs], b1 @ Wk[:, hs]])   # [2, 256]
        vb_a = np.stack([b2 @ Wv[:, hs], b1 @ Wv[:, hs]])    # [2, 256]
        wo_g = np.stack([Wo[hs, :][0:128], Wo[hs, :][128:256]])  # [2,128,DIM]
        in_maps.append(dict(
            xb=np.ascontiguousarray(x[b]).astype(bf),
            cb=np.ascontiguousarray(context[b]).astype(bf),
            wq=np.ascontiguousarray(wq_g).astype(bf),
            wk=np.ascontiguousarray(wk2).astype(bf),
            wv=np.ascontiguousarray(wv2).astype(bf),
            wo=np.ascontiguousarray(wo_g).astype(bf),
            cbq=np.ascontiguousarray(cbq_a.reshape(2, 128).T),
            cbk=np.ascontiguousarray(
                cbk_a.reshape(2, 2, 128).transpose(2, 0, 1)),
            vbv=np.ascontiguousarray(np.broadcast_to(
                vb_a.reshape(1, 2, HPC, 64), (128, 2, HPC, 64))).astype(bf),
            cmv=np.ascontiguousarray(
                np.asarray(context_mask[b], np.float32).reshape(16, 128).T
            ),
            tri01=tri,
            ident=np.eye(128, dtype=np.float32).astype(bf),
        ))
    return in_maps


_NC_CACHE = None


def kernel(**inputs) -> np.ndarray:
    global _NC_CACHE
    x = np.asarray(inputs["x"], np.float32)
    context = np.asarray(inputs["context"], np.float32)
    cm = np.asarray(inputs["context_mask"])
    g1 = np.asarray(inputs["g1"], np.float32)
    b1 = np.asarray(inputs["b1"], np.float32)
    g2 = np.asarray(inputs["g2"], np.float32)
    b2 = np.asarray(inputs["b2"], np.float32)
    Wq = np.asarray(inputs["Wq"], np.float32)
    Wkv = np.asarray(inputs["Wkv"], np.float32)
    Wo = np.asarray(inputs["Wo"], np.float32)
    bo = np.asarray(inputs["bo"], np.float32)

    if _NC_CACHE is None:
        _NC_CACHE = build_nc()
    nc = _NC_CACHE

    # The SPMD run dispatches through jax/PJRT on the axon backend; if the
    # caller pinned jax to cpu (common for reference computation), restore
    # the full platform list so the 8 NeuronCores are visible.
    import jax
    if len(jax.devices()) < 8:
        import os
        os.environ.pop("JAX_PLATFORMS", None)
        try:
            jax.config.update("jax_platforms", None)
        except Exception:
            pass
        try:
            from jax.extend import backend as _jxb
            _jxb.clear_backends()
        except Exception:
            from jax._src import xla_bridge as _xb
            _xb.backends.cache_clear()

    in_maps = make_in_maps(x, context, cm, g1, b1, g2, b2, Wq, Wkv, Wo)
    res = run_bass_kernel_spmd(nc, in_maps, core_ids=list(range(8))).results

    out = np.zeros((2, N, DIM), np.float32)
    for core in range(8):
        out[core // 4] += np.asarray(res[core]["out"], np.float32)
    out += bo
    return out
